# revision 50
# baseline (speedup 1.0000x reference)
"""Trainium2 Bass kernel for nn_ContrastByClassCalculator (MoCo-style
per-class-queue contrastive loss).

Math (reference):
    l_pos[n]  = q[n] . k[n]                                  # [N, 1]
    l_neg[n,:] = q[n] @ queue[cls_labels[n]]                 # [N, K]
    logits = concat([l_pos, l_neg], 1) / T                   # [N, 1+K]
    loss = mean_n( -log_softmax(logits)[n, 0] )

Sharding: the queue [C=100, D=128, K=2048] dominates memory traffic
(~105 MB), so we shard it over classes across the 8 cores (13 classes
each, with a 1-class overlap window for the 12-class cores). Each core
computes the full loss rows for the samples whose label falls in its
class range, reduces them to a scalar partial sum on device, and the
host adds the 8 partials and divides by N.

Per-core device program (SPMD, identical structure on all 8 cores):
  - 13 class slots, each padded to 32 sample rows, packed 4 per
    128-partition "group" (4 groups: 4+4+4+1 slots).
  - Per slot: DMA the class's queue slab [128, 2048] to SBUF, then 4
    matmuls (N=512) with the slot's packed q vectors [128, 32] as
    stationary -> PSUM group tile rows 32s..32s+31.
  - Per group: row-max on DVE, fused exp+row-sum on ACT (both read
    PSUM directly), combined with the positive logit (computed on DVE
    from packed q/k rows).
  - Tail: one Ln pass, per-row loss, validity mask, ones-vector matmul
    to reduce over partitions -> scalar partial.

QDT selects the matmul datatype for the l_neg GEMMs:
  - "f32"  : exact fp32 (PE runs 2 half-speed passes, 4 cyc/col)
  - "f32r" : fp32 data, single-pass reduced-precision mode (1 cyc/col)
  - "bf16" : queue+q cast to bf16 on host (halves HBM traffic,
             1 cyc/col).  Loss error stays ~1e-5 relative because the
             row-max subtraction cancels in log-softmax and per-row
             errors average out over N=512.
The positive logits and the whole softmax run in fp32 regardless.
"""

import os

import numpy as np

import concourse.bacc as bacc
import concourse.mybir as mybir
import concourse.tile as tile
from concourse import bass_utils

# Problem constants (hardcoded per contract; kernel.py must be self-contained)
N = 512
D = 128
C = 100
K = 2048
T = 0.07
INV_T = float(1.0 / T)

N_CORES = 8
SLOTS = 13           # class slots per core (4 cores own 13 classes, 4 own 12)
M_PAD = 32           # rows per slot (PE col-group granularity)
GROUP_SLOTS = [(0, 4), (4, 8), (8, 12), (12, 13)]
N_GROUPS = len(GROUP_SLOTS)
# slab DMA chunks: one dispatch costs ~0.7us on the serial HWDGE ring, so
# ship slabs in a few large transfers.  First chunk is a single slab so the
# first matmul can start as early as possible.  Group 3's single slab ships
# BEFORE group 2's chunk (and groups are processed 0,1,3,2) so that when
# the last chunk lands, only ONE group's softmax chain remains on the tail.
DMA_CHUNKS = [(0, 1), (1, 4), (12, 13), (4, 8), (8, 12)]
GROUP_ORDER = [0, 1, 3, 2]
FP32 = mybir.dt.float32
BF16 = mybir.dt.bfloat16
# class range end per core: 4 cores x 13 classes + 4 cores x 12 classes
CLASS_ENDS = [13, 26, 39, 52, 64, 76, 88, 100]

# Matmul/shipping dtype for the l_neg GEMMs.  bf16 halves HBM traffic (the
# memory-bound axis of this problem) and costs ~3.4e-5 relative loss error;
# set BASS_QDT=f32 for the exact (but ~1.5x slower) variant.
QDT = os.environ.get("BASS_QDT", "bf16")  # "bf16" | "f32" | "f32r"

# cpack column layout (fp32 columns); the matmul lhsT ("qt") ships as its
# own tensor so it can carry the matmul dtype end-to-end (walrus requires
# fp32r/bf16 operands to be typed at the producer, not bitcast at use).
QR_OFF = 0                            # [128, 512]  q rows, group-major
KR_OFF = QR_OFF + N_GROUPS * D        # [128, 512]  k rows, group-major
MSK_OFF = KR_OFF + N_GROUPS * D       # [128, 4]    row validity per group
ONE_OFF = MSK_OFF + N_GROUPS          # [128, 1]    all-ones column
CPACK_W = ONE_OFF + 1

# Results of the last hardware run (for test harnesses): BassKernelResults
last_run = None


def _build_nc():
    """Build the single-core SPMD Bass/Tile program.

    Bacc (not raw Bass): its finalize runs generate_event_semaphores,
    which splits multi-semaphore waits to satisfy the TRN2 1-wait-per-
    instruction constraint walrus enforces.
    """
    nc = bacc.Bacc("TRN2")

    mm_dt = {"f32": FP32, "f32r": mybir.dt.float32r, "bf16": BF16}[QDT]

    cpack_h = nc.dram_tensor("cpack", [D, CPACK_W], FP32, kind="ExternalInput")
    qt_h = nc.dram_tensor("qt", [D, SLOTS * M_PAD], mm_dt, kind="ExternalInput")
    slabs_h = nc.dram_tensor("slabs", [SLOTS, D, K], mm_dt, kind="ExternalInput")
    out_h = nc.dram_tensor("out", [1, 1], FP32, kind="ExternalOutput")

    AX = mybir.AxisListType
    AF = mybir.ActivationFunctionType

    with tile.TileContext(nc) as tc:
        with (
            tc.tile_pool(name="consts", bufs=1) as consts,
            tc.tile_pool(name="small", bufs=1) as small,
            tc.tile_pool(name="scr", bufs=2) as scr,
            tc.tile_pool(name="slab", bufs=1) as slab_pool,
            tc.tile_pool(name="esc", bufs=2) as esc_pool,
            tc.tile_pool(name="psum", bufs=2, space="PSUM") as psum_pool,
        ):
            # DMA dispatch order matters (FIFO per HWDGE ring): first slab
            # chunk, then the small qt, then cpack, then remaining chunks
            # alternating across the two rings.
            slab_tiles = {}  # slot -> (tile, col offset)
            for ci, (c0, c1) in enumerate(DMA_CHUNKS):
                st = slab_pool.tile([D, (c1 - c0) * K], mm_dt, tag=f"slab{c0}")
                nc.sync.dma_start(
                    out=st[:], in_=slabs_h[c0:c1].rearrange("n p k -> p n k")
                )
                for t in range(c0, c1):
                    slab_tiles[t] = (st, (t - c0) * K)
                if c0 == 0:
                    qt = consts.tile([D, SLOTS * M_PAD], mm_dt)
                    nc.sync.dma_start(out=qt[:], in_=qt_h[:])
                    # cpack rides early: the positive logits it carries gate
                    # each group's exp bias, and through that the PSUM slot
                    # releases — shipping it late cascades ~5us down the
                    # whole softmax pipeline.
                    cp = consts.tile([D, CPACK_W], FP32)
                    nc.sync.dma_start(out=cp[:], in_=cpack_h[:])

            # Warm the Exp spline table while the first DMAs stream.
            warm = small.tile([1, 1], FP32)
            nc.vector.memset(warm[:], 0.0)
            nc.scalar.activation(out=warm[:], in_=warm[:], func=AF.Exp)

            # Per-row stats, one column per group. Rows beyond a group's
            # active partitions keep the memset values, which yield a row
            # loss of exactly 0 (and are masked anyway).
            lpos = small.tile([128, N_GROUPS], FP32)
            nc.vector.memset(lpos[:], 0.0)
            nbias = small.tile([128, N_GROUPS], FP32)
            nc.vector.memset(nbias[:], 0.0)
            sneg = small.tile([128, N_GROUPS], FP32)
            nc.vector.memset(sneg[:], 0.0)

            for g in GROUP_ORDER:
                t0, t1 = GROUP_SLOTS[g]
                pg = 32 * (t1 - t0)
                col = slice(g, g + 1)

                # positive logit: per-row q.k (multiply then row-reduce)
                ttr = scr.tile([128, D], FP32, tag="ttr")
                nc.vector.tensor_mul(
                    ttr[0:pg],
                    cp[0:pg, QR_OFF + g * D:QR_OFF + (g + 1) * D],
                    cp[0:pg, KR_OFF + g * D:KR_OFF + (g + 1) * D],
                )
                nc.vector.reduce_sum(
                    out=lpos[0:pg, col], in_=ttr[0:pg], axis=AX.X
                )

                gps = psum_pool.tile([128, K], FP32, tag="gps")
                # Warm-keepers: PE's HAM clock-gate re-throttles to 1.2 GHz
                # after ~3.4us idle, and each group's matmuls wait on a DMA
                # chunk.  A few throwaway matmuls on always-resident data
                # (slab-0 chunk) keep the PE busy through the gap so the
                # real matmuls run at 2.4 GHz.  They write this group's own
                # zone [0:32, 0:512], which the first real matmul then
                # overwrites with start=True.
                if g != 0:
                    wsrc, _ = slab_tiles[0]
                    for _ in range(6):
                        nc.tensor.matmul(
                            out=gps[0:M_PAD, 0:512],
                            lhsT=qt[:, 0:M_PAD],
                            rhs=wsrc[:, 0:512],
                            start=True,
                            stop=True,
                            tile_position=(0, 0),
                        )
                for s in range(t1 - t0):
                    t = t0 + s
                    st, coff = slab_tiles[t]
                    for j in range(K // 512):
                        nc.tensor.matmul(
                            out=gps[32 * s:32 * s + 32, 512 * j:512 * (j + 1)],
                            lhsT=qt[:, M_PAD * t:M_PAD * (t + 1)],
                            rhs=st[:, coff + 512 * j:coff + 512 * (j + 1)],
                            start=True,
                            stop=True,
                            tile_position=(0, 32 * s),
                        )

                # row max over negatives; fold in the positive logit and the
                # -1/T exp-bias scale: nbias = -max(nm,lpos)/T.  The tiny
                # fold runs on the otherwise-idle GpSimd engine so it cannot
                # queue behind another group's 2.3us reduce on DVE (that
                # delay lands directly on the exp critical path at the tail).
                nm = scr.tile([128, 1], FP32, tag="nm")
                nc.vector.reduce_max(out=nm[0:pg], in_=gps[0:pg], axis=AX.X)
                nc.gpsimd.tensor_scalar(
                    out=nbias[0:pg, col],
                    in0=nm[0:pg],
                    scalar1=lpos[0:pg, col],
                    scalar2=-INV_T,
                    op0=mybir.AluOpType.max,
                    op1=mybir.AluOpType.mult,
                )

                # exp((l - rmax)/T) with fused row-sum on ACT
                esc = esc_pool.tile([128, K], FP32, tag="esc")
                nc.scalar.activation(
                    out=esc[0:pg],
                    in_=gps[0:pg],
                    func=AF.Exp,
                    bias=nbias[0:pg, col],
                    scale=INV_T,
                    accum_out=sneg[0:pg, col],
                )

            # Tail, all [128, 4]-wide: the positive-logit exp for every group
            # runs as ONE tiny ACT op: ppos = exp(lpos/T + nbias), then
            # stot = sneg + ppos, row_loss = log(stot) - (lpos/T + nbias),
            # masked, then partition-reduce via ones-vector matmul.
            pprep = small.tile([128, N_GROUPS], FP32)
            nc.vector.scalar_tensor_tensor(
                out=pprep[:], in0=lpos[:], scalar=INV_T, in1=nbias[:],
                op0=mybir.AluOpType.mult, op1=mybir.AluOpType.add,
            )
            ppos = small.tile([128, N_GROUPS], FP32)
            nc.scalar.activation(out=ppos[:], in_=pprep[:], func=AF.Exp)
            stot = small.tile([128, N_GROUPS], FP32)
            nc.vector.tensor_add(stot[:], sneg[:], ppos[:])
            lt = small.tile([128, N_GROUPS], FP32)
            nc.scalar.activation(out=lt[:], in_=stot[:], func=AF.Ln)
            rloss = small.tile([128, N_GROUPS], FP32)
            nc.vector.tensor_sub(rloss[:], lt[:], pprep[:])
            mrl = small.tile([128, N_GROUPS], FP32)
            nc.vector.tensor_mul(mrl[:], rloss[:], cp[:, MSK_OFF:MSK_OFF + N_GROUPS])

            fps = psum_pool.tile([128, K], FP32, tag="gps")
            nc.tensor.matmul(
                out=fps[0:1, 0:N_GROUPS],
                lhsT=cp[:, ONE_OFF:ONE_OFF + 1],
                rhs=mrl[:, 0:N_GROUPS],
                start=True,
                stop=True,
                tile_position=(0, 0),
            )
            osb = small.tile([1, 1], FP32)
            nc.vector.reduce_sum(out=osb[0:1], in_=fps[0:1, 0:N_GROUPS], axis=AX.X)
            nc.sync.dma_start(out=out_h[:], in_=osb[:])

    return nc


def _pack_inputs(q, k, queue, cls_labels):
    """Host-side packing: per-core slab windows + padded per-class q/k rows."""
    import ml_dtypes

    in_maps = []
    for i in range(N_CORES):
        end = CLASS_ENDS[i]
        own_start = CLASS_ENDS[i - 1] if i > 0 else 0
        w0 = end - SLOTS  # slab window start (may include 1 unowned class)

        cpack = np.zeros((D, CPACK_W), dtype=np.float32)
        cpack[:, ONE_OFF] = 1.0
        qt = np.zeros((D, SLOTS * M_PAD), dtype=np.float32)

        for t in range(SLOTS):
            c = w0 + t
            if c < own_start:
                continue  # overlap slot: slab read but no rows assigned
            rows = np.nonzero(cls_labels == c)[0]
            if len(rows) > M_PAD:
                raise ValueError(
                    f"class {c} has {len(rows)} samples > M_PAD={M_PAD}"
                )
            g, s = divmod(t, 4)
            for j, n in enumerate(rows):
                p = 32 * s + j
                qt[:, M_PAD * t + j] = q[n]
                cpack[p, QR_OFF + g * D:QR_OFF + (g + 1) * D] = q[n]
                cpack[p, KR_OFF + g * D:KR_OFF + (g + 1) * D] = k[n]
                cpack[p, MSK_OFF + g] = 1.0

        slabs = np.ascontiguousarray(queue[w0:end], dtype=np.float32)
        if QDT == "bf16":
            slabs = slabs.astype(ml_dtypes.bfloat16)
            qt = qt.astype(ml_dtypes.bfloat16)

        in_maps.append({"cpack": cpack, "qt": qt, "slabs": slabs})
    return in_maps


def kernel(q, k, queue, class_weights, cls_labels):
    global last_run
    q = np.asarray(q, dtype=np.float32)
    k = np.asarray(k, dtype=np.float32)
    queue = np.asarray(queue, dtype=np.float32)
    cls_labels = np.asarray(cls_labels).astype(np.int64)

    in_maps = _pack_inputs(q, k, queue, cls_labels)
    nc = _build_nc()
    if not nc.is_finalized():
        nc.finalize()  # runs Bacc passes: reg alloc + event-semaphore wait split

    trace = bool(os.environ.get("BASS_TRACE"))
    res = bass_utils.run_bass_kernel_spmd(
        nc, in_maps, list(range(N_CORES)), trace=trace
    )
    last_run = res

    partial = sum(float(r["out"][0, 0]) for r in res.results)
    return np.float32(partial / N)


# revision 51
# speedup vs baseline: 1.0249x; 1.0249x over previous
"""Trainium2 Bass kernel for nn_ContrastByClassCalculator (MoCo-style
per-class-queue contrastive loss).

Math (reference):
    l_pos[n]  = q[n] . k[n]                                  # [N, 1]
    l_neg[n,:] = q[n] @ queue[cls_labels[n]]                 # [N, K]
    logits = concat([l_pos, l_neg], 1) / T                   # [N, 1+K]
    loss = mean_n( -log_softmax(logits)[n, 0] )

Sharding: the queue [C=100, D=128, K=2048] dominates memory traffic
(~105 MB), so we shard it over classes across the 8 cores (13 classes
each, with a 1-class overlap window for the 12-class cores). Each core
computes the full loss rows for the samples whose label falls in its
class range, reduces them to a scalar partial sum on device, and the
host adds the 8 partials and divides by N.

Per-core device program (SPMD, identical structure on all 8 cores):
  - 13 class slots, each padded to 32 sample rows, packed 4 per
    128-partition "group" (4 groups: 4+4+4+1 slots).
  - Per slot: DMA the class's queue slab [128, 2048] to SBUF, then 4
    matmuls (N=512) with the slot's packed q vectors [128, 32] as
    stationary -> PSUM group tile rows 32s..32s+31.
  - Per group: row-max on DVE, fused exp+row-sum on ACT (both read
    PSUM directly), combined with the positive logit (computed on DVE
    from packed q/k rows).
  - Tail: one Ln pass, per-row loss, validity mask, ones-vector matmul
    to reduce over partitions -> scalar partial.

QDT selects the matmul datatype for the l_neg GEMMs:
  - "f32"  : exact fp32 (PE runs 2 half-speed passes, 4 cyc/col)
  - "f32r" : fp32 data, single-pass reduced-precision mode (1 cyc/col)
  - "bf16" : queue+q cast to bf16 on host (halves HBM traffic,
             1 cyc/col).  Loss error stays ~1e-5 relative because the
             row-max subtraction cancels in log-softmax and per-row
             errors average out over N=512.
The positive logits and the whole softmax run in fp32 regardless.
"""

import os

import numpy as np

import concourse.bacc as bacc
import concourse.mybir as mybir
import concourse.tile as tile
from concourse import bass_utils

# Problem constants (hardcoded per contract; kernel.py must be self-contained)
N = 512
D = 128
C = 100
K = 2048
T = 0.07
INV_T = float(1.0 / T)

N_CORES = 8
SLOTS = 13           # class slots per core (4 cores own 13 classes, 4 own 12)
M_PAD = 32           # rows per slot (PE col-group granularity)
GROUP_SLOTS = [(0, 4), (4, 8), (8, 12), (12, 13)]
N_GROUPS = len(GROUP_SLOTS)
# slab DMA chunks: one dispatch costs ~0.7us on the serial HWDGE ring, so
# ship slabs in a few large transfers.  First chunk is a single slab so the
# first matmul can start as early as possible.  Group 3's single slab ships
# BEFORE group 2's chunk (and groups are processed 0,1,3,2) so that when
# the last chunk lands, only ONE group's softmax chain remains on the tail.
DMA_CHUNKS = [(0, 1), (1, 4), (12, 13), (4, 8), (8, 12)]
GROUP_ORDER = [0, 1, 3, 2]
FP32 = mybir.dt.float32
BF16 = mybir.dt.bfloat16
# class range end per core: 4 cores x 13 classes + 4 cores x 12 classes
CLASS_ENDS = [13, 26, 39, 52, 64, 76, 88, 100]

# Matmul/shipping dtype for the l_neg GEMMs.  bf16 halves HBM traffic (the
# memory-bound axis of this problem) and costs ~3.4e-5 relative loss error;
# set BASS_QDT=f32 for the exact (but ~1.5x slower) variant.
QDT = os.environ.get("BASS_QDT", "bf16")  # "bf16" | "f32" | "f32r"

# cpack column layout (fp32 columns); the matmul lhsT ("qt") ships as its
# own tensor so it can carry the matmul dtype end-to-end (walrus requires
# fp32r/bf16 operands to be typed at the producer, not bitcast at use).
QR_OFF = 0                            # [128, 512]  q rows, group-major
KR_OFF = QR_OFF + N_GROUPS * D        # [128, 512]  k rows, group-major
MSK_OFF = KR_OFF + N_GROUPS * D       # [128, 4]    row validity per group
ONE_OFF = MSK_OFF + N_GROUPS          # [128, 1]    all-ones column
CPACK_W = ONE_OFF + 1

# Results of the last hardware run (for test harnesses): BassKernelResults
last_run = None


def _build_nc():
    """Build the single-core SPMD Bass/Tile program.

    Bacc (not raw Bass): its finalize runs generate_event_semaphores,
    which splits multi-semaphore waits to satisfy the TRN2 1-wait-per-
    instruction constraint walrus enforces.
    """
    nc = bacc.Bacc("TRN2")

    mm_dt = {"f32": FP32, "f32r": mybir.dt.float32r, "bf16": BF16}[QDT]

    cpack_h = nc.dram_tensor("cpack", [D, CPACK_W], FP32, kind="ExternalInput")
    qt_h = nc.dram_tensor("qt", [D, SLOTS * M_PAD], mm_dt, kind="ExternalInput")
    slabs_h = nc.dram_tensor("slabs", [SLOTS, D, K], mm_dt, kind="ExternalInput")
    out_h = nc.dram_tensor("out", [1, 1], FP32, kind="ExternalOutput")

    AX = mybir.AxisListType
    AF = mybir.ActivationFunctionType

    with tile.TileContext(nc) as tc:
        with (
            tc.tile_pool(name="consts", bufs=1) as consts,
            tc.tile_pool(name="small", bufs=1) as small,
            tc.tile_pool(name="scr", bufs=2) as scr,
            tc.tile_pool(name="slab", bufs=1) as slab_pool,
            tc.tile_pool(name="esc", bufs=2) as esc_pool,
            tc.tile_pool(name="psum", bufs=2, space="PSUM") as psum_pool,
        ):
            # DMA dispatch order matters (FIFO per HWDGE ring): first slab
            # chunk, then the small qt, then cpack, then remaining chunks
            # alternating across the two rings.
            slab_tiles = {}  # slot -> (tile, col offset)
            for ci, (c0, c1) in enumerate(DMA_CHUNKS):
                st = slab_pool.tile([D, (c1 - c0) * K], mm_dt, tag=f"slab{c0}")
                nc.sync.dma_start(
                    out=st[:], in_=slabs_h[c0:c1].rearrange("n p k -> p n k")
                )
                for t in range(c0, c1):
                    slab_tiles[t] = (st, (t - c0) * K)
                if c0 == 0:
                    qt = consts.tile([D, SLOTS * M_PAD], mm_dt)
                    nc.sync.dma_start(out=qt[:], in_=qt_h[:])
                    # cpack rides early: the positive logits it carries gate
                    # each group's exp bias, and through that the PSUM slot
                    # releases — shipping it late cascades ~5us down the
                    # whole softmax pipeline.
                    cp = consts.tile([D, CPACK_W], FP32)
                    nc.sync.dma_start(out=cp[:], in_=cpack_h[:])

            # Warm the Exp spline table while the first DMAs stream.
            warm = small.tile([1, 1], FP32)
            nc.vector.memset(warm[:], 0.0)
            nc.scalar.activation(out=warm[:], in_=warm[:], func=AF.Exp)

            # Per-row stats, one column per group. Rows beyond a group's
            # active partitions keep the memset values, which yield a row
            # loss of exactly 0 (and are masked anyway).
            lpos = small.tile([128, N_GROUPS], FP32)
            nc.vector.memset(lpos[:], 0.0)
            nbias = small.tile([128, N_GROUPS], FP32)
            nc.vector.memset(nbias[:], 0.0)
            sneg = small.tile([128, N_GROUPS], FP32)
            nc.vector.memset(sneg[:], 0.0)

            for g in GROUP_ORDER:
                t0, t1 = GROUP_SLOTS[g]
                pg = 32 * (t1 - t0)
                col = slice(g, g + 1)

                # positive logit: per-row q.k (multiply then row-reduce)
                ttr = scr.tile([128, D], FP32, tag="ttr")
                nc.vector.tensor_mul(
                    ttr[0:pg],
                    cp[0:pg, QR_OFF + g * D:QR_OFF + (g + 1) * D],
                    cp[0:pg, KR_OFF + g * D:KR_OFF + (g + 1) * D],
                )
                nc.vector.reduce_sum(
                    out=lpos[0:pg, col], in_=ttr[0:pg], axis=AX.X
                )

                gps = psum_pool.tile([128, K], FP32, tag="gps")
                for s in range(t1 - t0):
                    t = t0 + s
                    st, coff = slab_tiles[t]
                    for j in range(K // 512):
                        nc.tensor.matmul(
                            out=gps[32 * s:32 * s + 32, 512 * j:512 * (j + 1)],
                            lhsT=qt[:, M_PAD * t:M_PAD * (t + 1)],
                            rhs=st[:, coff + 512 * j:coff + 512 * (j + 1)],
                            start=True,
                            stop=True,
                            tile_position=(0, 32 * s),
                        )

                # row max over negatives; fold in the positive logit and the
                # -1/T exp-bias scale: nbias = -max(nm,lpos)/T.  The tiny
                # fold runs on the otherwise-idle GpSimd engine so it cannot
                # queue behind another group's 2.3us reduce on DVE (that
                # delay lands directly on the exp critical path at the tail).
                nm = scr.tile([128, 1], FP32, tag="nm")
                nc.vector.reduce_max(out=nm[0:pg], in_=gps[0:pg], axis=AX.X)
                nc.gpsimd.tensor_scalar(
                    out=nbias[0:pg, col],
                    in0=nm[0:pg],
                    scalar1=lpos[0:pg, col],
                    scalar2=-INV_T,
                    op0=mybir.AluOpType.max,
                    op1=mybir.AluOpType.mult,
                )

                # exp((l - rmax)/T) with fused row-sum on ACT
                esc = esc_pool.tile([128, K], FP32, tag="esc")
                nc.scalar.activation(
                    out=esc[0:pg],
                    in_=gps[0:pg],
                    func=AF.Exp,
                    bias=nbias[0:pg, col],
                    scale=INV_T,
                    accum_out=sneg[0:pg, col],
                )

            # Tail, all [128, 4]-wide: the positive-logit exp for every group
            # runs as ONE tiny ACT op: ppos = exp(lpos/T + nbias), then
            # stot = sneg + ppos, row_loss = log(stot) - (lpos/T + nbias),
            # masked, then partition-reduce via ones-vector matmul.
            pprep = small.tile([128, N_GROUPS], FP32)
            nc.vector.scalar_tensor_tensor(
                out=pprep[:], in0=lpos[:], scalar=INV_T, in1=nbias[:],
                op0=mybir.AluOpType.mult, op1=mybir.AluOpType.add,
            )
            ppos = small.tile([128, N_GROUPS], FP32)
            nc.scalar.activation(out=ppos[:], in_=pprep[:], func=AF.Exp)
            stot = small.tile([128, N_GROUPS], FP32)
            nc.vector.tensor_add(stot[:], sneg[:], ppos[:])
            lt = small.tile([128, N_GROUPS], FP32)
            nc.scalar.activation(out=lt[:], in_=stot[:], func=AF.Ln)
            rloss = small.tile([128, N_GROUPS], FP32)
            nc.vector.tensor_sub(rloss[:], lt[:], pprep[:])
            mrl = small.tile([128, N_GROUPS], FP32)
            nc.vector.tensor_mul(mrl[:], rloss[:], cp[:, MSK_OFF:MSK_OFF + N_GROUPS])

            fps = psum_pool.tile([128, K], FP32, tag="gps")
            nc.tensor.matmul(
                out=fps[0:1, 0:N_GROUPS],
                lhsT=cp[:, ONE_OFF:ONE_OFF + 1],
                rhs=mrl[:, 0:N_GROUPS],
                start=True,
                stop=True,
                tile_position=(0, 0),
            )
            osb = small.tile([1, 1], FP32)
            nc.vector.reduce_sum(out=osb[0:1], in_=fps[0:1, 0:N_GROUPS], axis=AX.X)
            nc.sync.dma_start(out=out_h[:], in_=osb[:])

    return nc


def _pack_inputs(q, k, queue, cls_labels):
    """Host-side packing: per-core slab windows + padded per-class q/k rows."""
    import ml_dtypes

    in_maps = []
    for i in range(N_CORES):
        end = CLASS_ENDS[i]
        own_start = CLASS_ENDS[i - 1] if i > 0 else 0
        w0 = end - SLOTS  # slab window start (may include 1 unowned class)

        cpack = np.zeros((D, CPACK_W), dtype=np.float32)
        cpack[:, ONE_OFF] = 1.0
        qt = np.zeros((D, SLOTS * M_PAD), dtype=np.float32)

        for t in range(SLOTS):
            c = w0 + t
            if c < own_start:
                continue  # overlap slot: slab read but no rows assigned
            rows = np.nonzero(cls_labels == c)[0]
            if len(rows) > M_PAD:
                raise ValueError(
                    f"class {c} has {len(rows)} samples > M_PAD={M_PAD}"
                )
            g, s = divmod(t, 4)
            for j, n in enumerate(rows):
                p = 32 * s + j
                qt[:, M_PAD * t + j] = q[n]
                cpack[p, QR_OFF + g * D:QR_OFF + (g + 1) * D] = q[n]
                cpack[p, KR_OFF + g * D:KR_OFF + (g + 1) * D] = k[n]
                cpack[p, MSK_OFF + g] = 1.0

        slabs = np.ascontiguousarray(queue[w0:end], dtype=np.float32)
        if QDT == "bf16":
            slabs = slabs.astype(ml_dtypes.bfloat16)
            qt = qt.astype(ml_dtypes.bfloat16)

        in_maps.append({"cpack": cpack, "qt": qt, "slabs": slabs})
    return in_maps


def kernel(q, k, queue, class_weights, cls_labels):
    global last_run
    q = np.asarray(q, dtype=np.float32)
    k = np.asarray(k, dtype=np.float32)
    queue = np.asarray(queue, dtype=np.float32)
    cls_labels = np.asarray(cls_labels).astype(np.int64)

    in_maps = _pack_inputs(q, k, queue, cls_labels)
    nc = _build_nc()
    if not nc.is_finalized():
        nc.finalize()  # runs Bacc passes: reg alloc + event-semaphore wait split

    trace = bool(os.environ.get("BASS_TRACE"))
    res = bass_utils.run_bass_kernel_spmd(
        nc, in_maps, list(range(N_CORES)), trace=trace
    )
    last_run = res

    partial = sum(float(r["out"][0, 0]) for r in res.results)
    return np.float32(partial / N)
